# revision 5
# baseline (speedup 1.0000x reference)
"""GRU autoencoder Trainium2 kernel (8 NeuronCores, data-parallel over batch).

Reference model (PyTorch-layout GRU):
  encoder GRU over T=1024 steps -> h_n -> enc_fc -> z [B,16]
  decoder GRU with constant input dec_fc(z), T steps -> hs -> out_fc -> recon

Strategy per core (B_local = 64):
  - hidden state kept transposed: hT [H=128 partitions, B=64]
  - gate preactivations accumulated in PSUM: a block matmul precomputes the
    input projections xp for 8 steps (biases folded via an appended ones-row
    on the host-prepped transposed input), then per-step W_hh matmuls
    accumulate on top (start=False).
  - all transposes / bias folding / output unshuffling done host-side in numpy.
"""

import os
import sys
import time

for _p in ("/opt/trn_rl_repo",):
    if _p not in sys.path:
        sys.path.insert(0, _p)

import ml_dtypes
import numpy as np

import concourse.bass as bass
import concourse.mybir as mybir
import concourse.tile as tile
from concourse import bacc
from concourse.bass_utils import run_bass_kernel_spmd

F32 = mybir.dt.float32
AF = mybir.ActivationFunctionType
ALU = mybir.AluOpType

NCORES = 8
B_FULL, T_FULL, D, H, L = 512, 1024, 5, 128, 16
B = B_FULL // NCORES  # 64 per core
TBLK = 8              # timesteps per xp block (TBLK*B = 512 = max fp32 moving N)


def build_nc(T=T_FULL, repeat=None, dt=F32):
    """Build + compile the per-core program. Returns nc."""
    DT = dt
    nc = bacc.Bacc("TRN2", target_bir_lowering=False, debug=False,
                   num_devices=NCORES)

    TB = T * B
    # inputs (host-prepped, replicated weights)
    d_xt = nc.declare_dram_parameter("xt", [D + 1, TB], DT, isOutput=False)
    d_wxp = nc.declare_dram_parameter("w_xp", [D + 1, 3 * H], DT, isOutput=False)
    d_wehh = nc.declare_dram_parameter("w_ehh", [H, 3 * H], DT, isOutput=False)
    d_vebhn = nc.declare_dram_parameter("v_ebhn", [H, 1], F32, isOutput=False)
    d_wefc = nc.declare_dram_parameter("w_efc", [H, L], DT, isOutput=False)
    d_vefcb = nc.declare_dram_parameter("v_efcb", [L, 1], F32, isOutput=False)
    d_wdfc = nc.declare_dram_parameter("w_dfc", [L + 1, H], DT, isOutput=False)
    d_wdxp = nc.declare_dram_parameter("w_dxp", [H, 3 * H], DT, isOutput=False)
    d_vdxb = nc.declare_dram_parameter("v_dxb", [H, 3], F32, isOutput=False)
    d_wdhh = nc.declare_dram_parameter("w_dhh", [H, 3 * H], DT, isOutput=False)
    d_vdbhn = nc.declare_dram_parameter("v_dbhn", [H, 1], F32, isOutput=False)
    d_wout = nc.declare_dram_parameter("w_out", [H, D], DT, isOutput=False)
    # outputs
    d_rec = nc.declare_dram_parameter("recont", [D, TB], F32, isOutput=True)
    d_zt = nc.declare_dram_parameter("zt", [L, B], F32, isOutput=True)

    nblk = T // TBLK

    with tile.TileContext(nc) as tc:
        with tc.tile_pool(name="singles", bufs=1) as singles, \
             tc.tile_pool(name="hstate", bufs=1) as hstate:
            # weights to SBUF once
            w_xp = singles.tile([D + 1, 3 * H], DT)
            nc.sync.dma_start(out=w_xp[:], in_=d_wxp[:])
            w_ehh = singles.tile([H, 3 * H], DT)
            nc.sync.dma_start(out=w_ehh[:], in_=d_wehh[:])
            v_ebhn = singles.tile([H, 1], F32)
            nc.sync.dma_start(out=v_ebhn[:], in_=d_vebhn[:])
            w_efc = singles.tile([H, L], DT)
            nc.sync.dma_start(out=w_efc[:], in_=d_wefc[:])
            v_efcb = singles.tile([L, 1], F32)
            nc.sync.dma_start(out=v_efcb[:], in_=d_vefcb[:])
            w_dfc = singles.tile([L + 1, H], DT)
            nc.sync.dma_start(out=w_dfc[:], in_=d_wdfc[:])
            w_dxp = singles.tile([H, 3 * H], DT)
            nc.sync.dma_start(out=w_dxp[:], in_=d_wdxp[:])
            v_dxb = singles.tile([H, 3], F32)
            nc.sync.dma_start(out=v_dxb[:], in_=d_vdxb[:])
            w_dhh = singles.tile([H, 3 * H], DT)
            nc.sync.dma_start(out=w_dhh[:], in_=d_wdhh[:])
            v_dbhn = singles.tile([H, 1], F32)
            nc.sync.dma_start(out=v_dbhn[:], in_=d_vdbhn[:])
            w_out = singles.tile([H, D], DT)
            nc.sync.dma_start(out=w_out[:], in_=d_wout[:])

            def body():
                h_tiles = [hstate.tile([H, B], DT, tag="hA", name="hA"),
                           hstate.tile([H, B], DT, tag="hB", name="hB")]
                nc.vector.memset(h_tiles[0][:], 0.0)
                h_cur = h_tiles[0]

                # ---------------- encoder ----------------
                with tc.tile_pool(name="p_rz", bufs=2, space="PSUM") as p_rz, \
                     tc.tile_pool(name="p_nx", bufs=2, space="PSUM") as p_nx, \
                     tc.tile_pool(name="p_hpn", bufs=2, space="PSUM") as p_hpn, \
                     tc.tile_pool(name="xt_pool", bufs=3) as xt_pool, \
                     tc.tile_pool(name="ework", bufs=3) as work:
                    for blk in range(nblk):
                        xt_t = xt_pool.tile([D + 1, TBLK * B], DT)
                        nc.sync.dma_start(
                            out=xt_t[:],
                            in_=d_xt[:, blk * TBLK * B:(blk + 1) * TBLK * B])
                        prz = p_rz.tile([H, 2, TBLK * B], F32)
                        pnx = p_nx.tile([H, TBLK * B], F32)
                        nc.tensor.matmul(prz[:, 0, :], w_xp[:, 0:H], xt_t[:],
                                         start=True, stop=False)
                        nc.tensor.matmul(prz[:, 1, :], w_xp[:, H:2 * H], xt_t[:],
                                         start=True, stop=False)
                        nc.tensor.matmul(pnx[:, :], w_xp[:, 2 * H:3 * H], xt_t[:],
                                         start=True, stop=True)
                        for t8 in range(TBLK):
                            t = blk * TBLK + t8
                            sl = slice(t8 * B, (t8 + 1) * B)
                            nc.tensor.matmul(prz[:, 0, sl], w_ehh[:, 0:H],
                                             h_cur[:], start=False, stop=True)
                            nc.tensor.matmul(prz[:, 1, sl], w_ehh[:, H:2 * H],
                                             h_cur[:], start=False, stop=True)
                            pn = p_hpn.tile([H, B], F32)
                            nc.tensor.matmul(pn[:], w_ehh[:, 2 * H:3 * H],
                                             h_cur[:], start=True, stop=True)
                            rz = work.tile([H, 2, B], DT, tag="rz")
                            nc.scalar.activation(rz[:], prz[:, :, sl], AF.Sigmoid)
                            t1 = work.tile([H, B], DT, tag="t1")
                            nc.vector.scalar_tensor_tensor(
                                t1[:], pn[:], v_ebhn[:], rz[:, 0, :],
                                ALU.add, ALU.mult)
                            t2 = work.tile([H, B], DT, tag="t2")
                            nc.vector.tensor_add(t2[:], t1[:], pnx[:, sl])
                            nn = work.tile([H, B], DT, tag="nn")
                            nc.scalar.activation(nn[:], t2[:], AF.Tanh)
                            dd = work.tile([H, B], DT, tag="dd")
                            nc.vector.tensor_sub(dd[:], h_cur[:], nn[:])
                            ee = work.tile([H, B], DT, tag="ee")
                            nc.vector.tensor_mul(ee[:], dd[:], rz[:, 1, :])
                            h_nxt = h_tiles[(t + 1) % 2]
                            nc.vector.tensor_add(h_nxt[:], nn[:], ee[:])
                            h_cur = h_nxt

                # ---------------- latent + decoder ----------------
                with tc.tile_pool(name="p_misc", bufs=2, space="PSUM") as p_misc, \
                     tc.tile_pool(name="p_dhp", bufs=2, space="PSUM") as p_dhp, \
                     tc.tile_pool(name="p_dhpn", bufs=2, space="PSUM") as p_dhpn, \
                     tc.tile_pool(name="p_rec", bufs=2, space="PSUM") as p_rec, \
                     tc.tile_pool(name="dstate", bufs=1) as dstate, \
                     tc.tile_pool(name="hs_pool", bufs=2) as hs_pool, \
                     tc.tile_pool(name="dwork", bufs=3) as work:
                    # z = enc_fc @ h + b  (transposed: [L, B])
                    pz = p_misc.tile([L, B], F32, tag="m")
                    nc.tensor.matmul(pz[:], w_efc[:], h_cur[:],
                                     start=True, stop=True)
                    zaug = dstate.tile([32, B], DT, tag="zaug")
                    nc.vector.memset(zaug[:], 1.0)
                    nc.scalar.activation(zaug[0:L, :], pz[:], AF.Identity,
                                         bias=v_efcb[:])
                    zout = dstate.tile([L, B], F32, tag="zout")
                    nc.scalar.activation(zout[:], pz[:], AF.Identity,
                                         bias=v_efcb[:])
                    nc.sync.dma_start(out=d_zt[:], in_=zout[:])

                    # h_dec_in = dec_fc @ z + b (bias folded via zaug ones row)
                    pd = p_misc.tile([H, B], F32, tag="m")
                    nc.tensor.matmul(pd[:], w_dfc[:], zaug[0:L + 1, :],
                                     start=True, stop=True)
                    hdec = dstate.tile([H, B], DT, tag="hdec")
                    nc.scalar.activation(hdec[:], pd[:], AF.Copy)

                    # xp_dec per gate (constant across steps)
                    xpd_rz = dstate.tile([H, 2, B], DT, tag="xpd_rz")
                    xpd_n = dstate.tile([H, B], DT, tag="xpd_n")
                    for g in range(3):
                        pg = p_misc.tile([H, B], F32, tag="m")
                        nc.tensor.matmul(pg[:], w_dxp[:, g * H:(g + 1) * H],
                                         hdec[:], start=True, stop=True)
                        dst = xpd_n[:] if g == 2 else xpd_rz[:, g, :]
                        nc.scalar.activation(dst, pg[:], AF.Identity,
                                             bias=v_dxb[:, g:g + 1])

                    h_cur2 = h_tiles[0]
                    nc.vector.memset(h_cur2[:], 0.0)
                    h_ap = h_cur2[:]
                    for blk in range(nblk):
                        hsr = hs_pool.tile([H, TBLK * B], DT, tag="hsr")
                        for t8 in range(TBLK):
                            sl = slice(t8 * B, (t8 + 1) * B)
                            phr = p_dhp.tile([H, 2, B], F32, tag="phr")
                            nc.tensor.matmul(phr[:, 0, :], w_dhh[:, 0:H],
                                             h_ap, start=True, stop=True)
                            nc.tensor.matmul(phr[:, 1, :], w_dhh[:, H:2 * H],
                                             h_ap, start=True, stop=True)
                            pn = p_dhpn.tile([H, B], F32, tag="pnd")
                            nc.tensor.matmul(pn[:], w_dhh[:, 2 * H:3 * H],
                                             h_ap, start=True, stop=True)
                            srz = work.tile([H, 2, B], DT, tag="srz")
                            nc.vector.tensor_add(srz[:], phr[:], xpd_rz[:])
                            rz = work.tile([H, 2, B], DT, tag="rz")
                            nc.scalar.activation(rz[:], srz[:], AF.Sigmoid)
                            t1 = work.tile([H, B], DT, tag="t1")
                            nc.vector.scalar_tensor_tensor(
                                t1[:], pn[:], v_dbhn[:], rz[:, 0, :],
                                ALU.add, ALU.mult)
                            t2 = work.tile([H, B], DT, tag="t2")
                            nc.vector.tensor_add(t2[:], t1[:], xpd_n[:])
                            nn = work.tile([H, B], DT, tag="nn")
                            nc.scalar.activation(nn[:], t2[:], AF.Tanh)
                            dd = work.tile([H, B], DT, tag="dd")
                            nc.vector.tensor_sub(dd[:], h_ap, nn[:])
                            ee = work.tile([H, B], DT, tag="ee")
                            nc.vector.tensor_mul(ee[:], dd[:], rz[:, 1, :])
                            nc.vector.tensor_add(hsr[:, sl], nn[:], ee[:])
                            h_ap = hsr[:, sl]
                        prec = p_rec.tile([D, TBLK * B], F32, tag="prec")
                        nc.tensor.matmul(prec[:], w_out[:], hsr[:],
                                         start=True, stop=True)
                        rsb = work.tile([D, TBLK * B], F32, tag="rsb")
                        nc.scalar.activation(rsb[:], prec[:], AF.Copy)
                        nc.sync.dma_start(
                            out=d_rec[:, blk * TBLK * B:(blk + 1) * TBLK * B],
                            in_=rsb[:])

            if repeat:
                with tc.For_i(0, repeat, 1):
                    body()
            else:
                body()

    nc.compile()
    return nc


def _np(dt):
    return np.float32


def prep_inputs(x, enc_W_ih, enc_W_hh, enc_b_ih, enc_b_hh, enc_fc_W, enc_fc_b,
                dec_fc_W, dec_fc_b, dec_W_ih, dec_W_hh, dec_b_ih, dec_b_hh,
                out_W, out_b, T=T_FULL, dt=F32):
    """Host-side prep: per-core in_maps (list of dicts)."""
    cast = np.float32 if dt == F32 else ml_dtypes.bfloat16
    f32 = np.float32

    def gate(w, g):
        return w[g * H:(g + 1) * H]

    # xp lhsT per gate with folded biases (ones-row trick)
    cols = []
    for g in range(3):
        b = enc_b_ih[g * H:(g + 1) * H].copy()
        if g < 2:
            b = b + enc_b_hh[g * H:(g + 1) * H]
        wg = np.concatenate([gate(enc_W_ih, g), b[:, None]], axis=1)  # [H, D+1]
        cols.append(wg.T)  # [D+1, H]
    w_xp = np.concatenate(cols, axis=1).astype(cast)  # [D+1, 3H]

    w_ehh = np.concatenate([gate(enc_W_hh, g).T for g in range(3)],
                           axis=1).astype(cast)  # [H, 3H]
    v_ebhn = enc_b_hh[2 * H:3 * H][:, None].astype(f32)
    w_efc = enc_fc_W.T.astype(cast)  # [H, L]
    v_efcb = enc_fc_b[:, None].astype(f32)
    w_dfc = np.concatenate([dec_fc_W, dec_fc_b[:, None]],
                           axis=1).T.astype(cast)  # [L+1, H]
    w_dxp = np.concatenate([gate(dec_W_ih, g).T for g in range(3)],
                           axis=1).astype(cast)  # [H, 3H]
    bxr = dec_b_ih[0:H] + dec_b_hh[0:H]
    bxz = dec_b_ih[H:2 * H] + dec_b_hh[H:2 * H]
    bxn = dec_b_ih[2 * H:3 * H]
    v_dxb = np.stack([bxr, bxz, bxn], axis=1).astype(f32)  # [H, 3]
    w_dhh = np.concatenate([gate(dec_W_hh, g).T for g in range(3)],
                           axis=1).astype(cast)  # [H, 3H]
    v_dbhn = dec_b_hh[2 * H:3 * H][:, None].astype(f32)
    w_out = out_W.T.astype(cast)  # [H, D]

    shared = dict(w_xp=w_xp, w_ehh=w_ehh, v_ebhn=v_ebhn, w_efc=w_efc,
                  v_efcb=v_efcb, w_dfc=w_dfc, w_dxp=w_dxp, v_dxb=v_dxb,
                  w_dhh=w_dhh, v_dbhn=v_dbhn, w_out=w_out)

    in_maps = []
    for c in range(NCORES):
        xc = x[c * B:(c + 1) * B, :T, :]            # [B, T, D]
        xt = xc.transpose(2, 1, 0).reshape(D, T * B)  # [D, T*B], col = t*B+b
        xt = np.concatenate([xt, np.ones((1, T * B), xt.dtype)], axis=0)
        m = dict(shared)
        m["xt"] = np.ascontiguousarray(xt.astype(cast))
        in_maps.append(m)
    return in_maps


def assemble_outputs(results, out_b, T=T_FULL):
    recon = np.empty((B_FULL, T, D), np.float32)
    z = np.empty((B_FULL, L), np.float32)
    for c in range(NCORES):
        rt = results[c]["recont"].reshape(D, T, B)     # [D, T, B]
        recon[c * B:(c + 1) * B] = rt.transpose(2, 1, 0)
        z[c * B:(c + 1) * B] = results[c]["zt"].T
    recon += out_b[None, None, :].astype(np.float32)
    return recon, z


_NC_CACHE = {}


KERNEL_DT = F32


def kernel(**inputs):
    key = "main%s" % KERNEL_DT
    if key not in _NC_CACHE:
        _NC_CACHE[key] = build_nc(dt=KERNEL_DT)
    nc = _NC_CACHE[key]
    in_maps = prep_inputs(dt=KERNEL_DT,
                          **{k: np.asarray(v) for k, v in inputs.items()})
    res = run_bass_kernel_spmd(nc, in_maps, list(range(NCORES)))
    return assemble_outputs(res.results, np.asarray(inputs["out_b"]))
